# revision 6
# baseline (speedup 1.0000x reference)
"""Trainium2 Bass kernel for nn_DecoderSplatting (v3).

Per-pixel gaussian-splat decoding over (8 views, 480x640), one view per
NeuronCore (8 cores, SPMD).  Key design (vs the v1 baseline at 460us
NTFF / ~7.5k DMA descriptors):

- HOST-side reformat: input per view -> rin[4, 120, 14*640] (chunk-major,
  partition p = h-row 120k+p, channels planar in the free dim) so each
  per-chunk DMA is one 35840-byte contiguous run per partition = 120
  descriptors.  Output -> rout[4, 120, 15*640] fp16, planar channel
  layout (contiguous stores, f32->f16 cast in the engine write path,
  host converts/permutes back).  gx/gy/consts merged into one [120, 676]
  tensor.  Total ~1.1k descriptors and ~26.6MB of HBM traffic.
- Engine balance (HW-measured costs): ACT does transcendentals + the
  per-partition-scalar Copy ops (scale/bias APs); DVE does the
  scalar-pointer mult-add chains and min/max/is_equal (Pool rejects
  those ALU ops); GpSimd does plain mult/add/sub tensor-tensor work.
- Stride-0 broadcast APs fuse the quat output scaling into one 4W-wide
  op and the means*sfac into one 3W-wide op.

Math (validated against the jax reference, rel ~7e-3 dominated by means
cancellation, same as baseline):
- e = [sig(ox)+gx, r11*sig(oy)+gy, c2]; depth = 1/(K1*sig(disp)+K2)
  means = t + (R@e) * exp(-0.5*(2*(ln(K2*E+K1+K2)-ln(1+E)) + ln|e|^2)),
  E = exp(-disp)  (the 1/A00 scale of inv(K) cancels in e/|e|)
- world quat wq = M_E @ q_cam (4x4 const per view), normalized, sign
  flipped iff min(wq)^2 == max(wq_i^2) (scipy/Shepperd pivot sign).
- All ACT transcendentals use only {Exp, Ln} => a single activation
  table set (natural_log_exp_and_others), no table thrash.
"""

import sys

import numpy as np

try:
    import concourse.bass as bass
except ImportError:  # pragma: no cover
    sys.path.insert(0, "/opt/trn_rl_repo")
    import concourse.bass as bass

import concourse.bacc as bacc

import concourse.mybir as mybir
from concourse.tile import TileContext

F32 = mybir.dt.float32
F16 = mybir.dt.float16
Alu = mybir.AluOpType
Act = mybir.ActivationFunctionType

NEAR, FAR = 0.05, 20.0
K1 = float(1.0 / NEAR - 1.0 / FAR)
K2 = float(1.0 / FAR)

V = 8
C = 14
H = 480
W = 640
P = 120          # partitions per chunk
NCHUNK = H // P  # 4
NCST = 32
CFW = W + NCHUNK + NCST  # gx | gy columns | scalar consts

# input channel permutation (raw -> kernel order)
#   raw:  [r,g,b, disp, opac, s0,s1,s2, qx,qy,qz,qw, ox,oy]
#   kern: [r,g,b, s0,s1,s2, disp, opac, qx,qy,qz,qw, ox,oy]
IN_PERM = [0, 1, 2, 5, 6, 7, 3, 4, 8, 9, 10, 11, 12, 13]
# output slot s in the kernel -> channel in the reference layout
SLOT_TO_REF = [4, 5, 6, 8, 9, 10, 7, 3, 0, 1, 2, 11, 12, 13, 14]

_CACHE = {}


class _CoveringSetBacc(bacc.Bacc):
    """Bacc whose act-table-load pass collapses to one covering table set.

    The stock pass assigns each activation the *first* table set containing
    its function (Exp -> exp_and_others, Ln -> natural_log), which ping-pongs
    a ~2.7us table load before nearly every activation.  All functions used
    here live in natural_log_exp_and_others, so rewrite every load to that
    covering set and drop the duplicates (the loads carry no sync info).
    """

    def insert_act_table_loads(self):
        super().insert_act_table_loads()
        from concourse.hw_specs import get_activation_tables

        tables = list(get_activation_tables(self.m.arch).items())
        used = set()
        for b in self.main_func.blocks:
            for i in b.instructions:
                if isinstance(i, mybir.InstActivation):
                    used.add(i.func)
        cover = None
        for idx, (_, funcs) in enumerate(tables):
            if used <= funcs:
                cover = idx
                break
        if cover is None:
            return
        for b in self.main_func.blocks:
            seen = False
            keep = []
            for i in b.instructions:
                if isinstance(i, mybir.InstLoadActFuncSet):
                    if seen:
                        continue
                    i.act_func_set_id = cover
                    seen = True
                keep.append(i)
            b.instructions[:] = keep


def _build_nc():
    nc = _CoveringSetBacc()
    rin = nc.dram_tensor("rin", [NCHUNK, P, C * W], F32, kind="ExternalInput")
    cfu = nc.dram_tensor("cfu", [P, CFW], F32, kind="ExternalInput")
    rout = nc.dram_tensor("rout", [NCHUNK, P, 15 * W], F16,
                          kind="ExternalOutput")

    va = nc.vector
    ae = nc.scalar
    ge = nc.gpsimd

    with TileContext(nc) as tc:
        with (
            tc.tile_pool(name="inp", bufs=2) as in_pool,
            tc.tile_pool(name="outp", bufs=2) as out_pool,
            tc.tile_pool(name="scr", bufs=1) as scr_pool,
            tc.tile_pool(name="consts", bufs=1) as cst_pool,
        ):
            cf = cst_pool.tile([P, CFW], F32, tag="cf", name="cf")
            nc.sync.dma_start(out=cf[:], in_=cfu[:])
            gx = cf[:, 0:W]

            def GY(k):
                return cf[:, W + k:W + k + 1]

            def CST(i):
                return cf[:, W + NCHUNK + i:W + NCHUNK + i + 1]

            # persistent scratch (single buffer, reused across chunks)
            T6 = scr_pool.tile([P, 6 * W], F32, tag="T6", name="T6")
            W4 = scr_pool.tile([P, 4 * W], F32, tag="W4", name="W4")
            E2 = scr_pool.tile([P, 2 * W], F32, tag="E2", name="E2")
            L2 = scr_pool.tile([P, 2 * W], F32, tag="L2", name="L2")
            XY = scr_pool.tile([P, 2 * W], F32, tag="XY", name="XY")
            E01 = scr_pool.tile([P, 2 * W], F32, tag="E01", name="E01")
            S5 = scr_pool.tile([P, 5 * W], F32, tag="S5", name="S5")
            W3 = scr_pool.tile([P, 3 * W], F32, tag="W3", name="W3")

            def sl(t, a, b):
                return t[:, a * W:b * W]

            la, sfac, smn, isv, wb = (sl(S5, i, i + 1) for i in range(5))
            sfacb = sl(S5, 1, 2).rearrange("p (o w) -> p o w", o=1) \
                .broadcast_to((P, 3, W))
            isvb = sl(S5, 3, 4).rearrange("p (o w) -> p o w", o=1) \
                .broadcast_to((P, 4, W))

            for k in range(NCHUNK):
                ein = nc.sync if k % 2 == 0 else nc.scalar
                eout = nc.scalar if k % 2 == 0 else nc.sync
                IT = in_pool.tile([P, C * W], F32, tag="IT", name="IT")
                ein.dma_start(out=IT[:], in_=rin[k])
                OT = out_pool.tile([P, 15 * W], F16, tag="OT", name="OT")

                def it(a, b):
                    return IT[:, a * W:b * W]

                def ot(a, b):
                    return OT[:, a * W:b * W]

                # --- softplus(rgb+scales), 6 channels wide -> slots 0..5
                ae.activation(T6[:], it(0, 6), Act.Exp)
                ae.activation(ot(0, 6), T6[:], Act.Ln, bias=1.0)
                va.tensor_scalar(ot(3, 6), ot(3, 6), CST(14), None, Alu.mult)

                # --- disp/opacity: E = exp(-x), la = ln(K2*E+K1+K2),
                #     L = ln(1+E), opac = exp(-L1)
                ae.activation(E2[:], it(6, 8), Act.Exp, scale=-1.0)
                ae.activation(la, sl(E2, 0, 1), Act.Ln, scale=K2, bias=CST(31))
                ae.activation(L2[:], E2[:], Act.Ln, bias=1.0)
                ae.activation(ot(6, 7), sl(L2, 1, 2), Act.Exp, scale=-1.0)

                # --- xy sigmoid chain (in place), then ray e0/e1
                ae.activation(XY[:], it(12, 14), Act.Exp, scale=-1.0)
                ae.activation(XY[:], XY[:], Act.Ln, bias=1.0)
                ae.activation(XY[:], XY[:], Act.Exp, scale=-1.0)
                ge.tensor_tensor(sl(E01, 0, 1), sl(XY, 0, 1), gx, Alu.add)
                va.tensor_scalar(sl(E01, 1, 2), sl(XY, 1, 2), CST(1), GY(k),
                                 Alu.mult, Alu.add)

                # --- |e|^2 and ln of it (XY dead -> reuse)
                ge.tensor_tensor(XY[:], E01[:], E01[:], Alu.mult)
                ge.tensor_tensor(sl(XY, 0, 1), sl(XY, 0, 1), sl(XY, 1, 2),
                                 Alu.add)
                ae.activation(sl(XY, 1, 2), sl(XY, 0, 1), Act.Ln, bias=CST(0))

                # --- ld = la - ln(1+E); arg = 2*ld + ln|e|^2; sfac
                ge.tensor_tensor(la, la, sl(L2, 0, 1), Alu.subtract)
                va.scalar_tensor_tensor(la, la, 2.0, sl(XY, 1, 2),
                                        Alu.mult, Alu.add)
                ae.activation(sfac, la, Act.Exp, scale=-0.5)

                # --- quat matvec: wq_i = sum_j M[i][j]*q_j
                #     (start on ACT via Copy-with-scale, accumulate on DVE)
                for i in range(4):
                    wqi = sl(W4, i, i + 1)
                    ae.activation(wqi, it(8, 9), Act.Copy,
                                  scale=CST(15 + 4 * i))
                    for j in range(1, 4):
                        va.scalar_tensor_tensor(wqi, it(8 + j, 9 + j),
                                                CST(15 + 4 * i + j), wqi,
                                                Alu.mult, Alu.add)

                # --- means: m_i = (e0*Ri0 + (e1*Ri1 + Ri2c2))*sfac + t_i
                for i in range(3):
                    va.tensor_scalar(sl(W3, i, i + 1), sl(E01, 1, 2),
                                     CST(5 + i), CST(8 + i),
                                     Alu.mult, Alu.add)
                    va.scalar_tensor_tensor(sl(W3, i, i + 1), sl(E01, 0, 1),
                                            CST(2 + i), sl(W3, i, i + 1),
                                            Alu.mult, Alu.add)
                ge.tensor_tensor(
                    W3[:].rearrange("p (i w) -> p i w", i=3),
                    W3[:].rearrange("p (i w) -> p i w", i=3),
                    sfacb, Alu.mult)
                for i in range(3):
                    ae.activation(ot(8 + i, 9 + i), sl(W3, i, i + 1),
                                  Act.Identity, bias=CST(11 + i))

                # --- quat norm + sign fix
                # squares into dead IT[8:12]; m12 into dead IT[12:14]
                # (Pool TT supports only add/sub/mult -> min/max/is_equal
                #  run on DVE)
                ge.tensor_tensor(it(8, 12), W4[:], W4[:], Alu.mult)
                va.tensor_tensor(it(12, 14), it(8, 10), it(10, 12), Alu.max)
                ge.tensor_tensor(it(8, 10), it(8, 10), it(10, 12), Alu.add)
                ge.tensor_tensor(it(8, 9), it(8, 9), it(9, 10), Alu.add)
                va.tensor_tensor(it(12, 13), it(12, 13), it(13, 14), Alu.max)
                va.tensor_tensor(it(10, 12), sl(W4, 0, 2), sl(W4, 2, 4),
                                 Alu.min)
                va.tensor_tensor(it(10, 11), it(10, 11), it(11, 12), Alu.min)
                ge.tensor_tensor(smn, it(10, 11), it(10, 11), Alu.mult)
                ae.activation(it(9, 10), it(8, 9), Act.Ln)
                ae.activation(isv, it(9, 10), Act.Exp, scale=-0.5)
                va.tensor_tensor(smn, smn, it(12, 13), Alu.is_equal)
                # isv *= sign: isv - 2*isv*iseq
                ge.tensor_tensor(wb, isv, smn, Alu.mult)
                va.scalar_tensor_tensor(isv, wb, -2.0, isv,
                                        Alu.mult, Alu.add)
                va.tensor_tensor(
                    OT[:, 11 * W:15 * W].rearrange("p (i w) -> p i w", i=4),
                    W4[:].rearrange("p (i w) -> p i w", i=4),
                    isvb, Alu.mult)

                ge.memset(ot(7, 8), 1.0)

                eout.dma_start(out=rout[k], in_=OT[:])
    nc.finalize()
    return nc


def _mat_to_quat_wxyz(m):
    m = np.asarray(m, np.float64)
    m00, m01, m02 = m[0, 0], m[0, 1], m[0, 2]
    m10, m11, m12 = m[1, 0], m[1, 1], m[1, 2]
    m20, m21, m22 = m[2, 0], m[2, 1], m[2, 2]
    tr = m00 + m11 + m22
    qs = [
        np.array([m21 - m12, 1 + m00 - m11 - m22, m01 + m10, m02 + m20]),
        np.array([m02 - m20, m01 + m10, 1 + m11 - m00 - m22, m12 + m21]),
        np.array([m10 - m01, m02 + m20, m12 + m21, 1 + m22 - m00 - m11]),
        np.array([1 + tr, m21 - m12, m02 - m20, m10 - m01]),
    ]
    q = qs[int(np.argmax([m00, m11, m22, tr]))]
    return q / np.linalg.norm(q)


def _per_view_inputs(raw_v, E, K):
    """Host-side reformat + per-view constants -> the in_map for one core."""
    A = np.linalg.inv(K.astype(np.float32))
    a00 = float(A[0, 0])
    assert a00 > 0
    assert abs(A[0, 1]) < 1e-6 * a00 and abs(A[1, 0]) < 1e-6 * a00
    assert abs(A[2, 0]) < 1e-9 and abs(A[2, 1]) < 1e-9
    assert np.allclose(E[3], [0, 0, 0, 1], atol=1e-6)
    R = E[:3, :3].astype(np.float64)
    t = E[:3, 3].astype(np.float64)
    c2 = float(A[2, 2]) / a00
    r11 = float(A[1, 1]) / a00
    mult = float(np.linalg.inv(K[:2, :2].astype(np.float32)).sum())

    ew, ex, ey, ez = _mat_to_quat_wxyz(R)
    M = np.array([
        [-ex, -ey, -ez, ew],
        [ew, -ez, ey, ex],
        [ez, ew, -ex, ey],
        [-ey, ex, ew, ez],
    ], np.float64)

    cstv = np.zeros(NCST, np.float64)
    cstv[0] = c2 * c2
    cstv[1] = r11
    for i in range(3):
        cstv[2 + i] = R[i, 0]
        cstv[5 + i] = R[i, 1]
        cstv[8 + i] = R[i, 2] * c2
        cstv[11 + i] = t[i]
    cstv[14] = mult
    cstv[15:31] = M.reshape(-1)
    cstv[31] = K1 + K2   # bias for ln(k2*E + (k1+k2))

    xs = np.arange(W, dtype=np.float32)
    gxrow = (xs - np.float32(0.5)) + np.float32(float(A[0, 2]) / a00)
    ys = np.arange(H, dtype=np.float32)
    gycol = np.float32(r11) * (ys - np.float32(0.5)) + \
        np.float32(float(A[1, 2]) / a00)
    gyt = gycol.reshape(NCHUNK, P).T  # [P, NCHUNK]

    cfu = np.empty((P, CFW), np.float32)
    cfu[:, 0:W] = gxrow[None, :]
    cfu[:, W:W + NCHUNK] = gyt
    cfu[:, W + NCHUNK:] = cstv.astype(np.float32)[None, :]

    # [14, 480, 640] -> perm channels -> [4, 120, 14, 640] -> flat free dim
    rin = np.ascontiguousarray(
        raw_v[IN_PERM].reshape(C, NCHUNK, P, W).transpose(1, 2, 0, 3)
    ).reshape(NCHUNK, P, C * W)

    return {"rin": rin, "cfu": cfu}


def kernel(raw_gaussians, extrinsics, intrinsics, _trace=False,
           _trace_kwargs=None):
    raw_gaussians = np.asarray(raw_gaussians, np.float32)
    extrinsics = np.asarray(extrinsics, np.float32)
    intrinsics = np.asarray(intrinsics, np.float32)
    b, v, c, h, w = raw_gaussians.shape
    assert (b, v, c, h, w) == (1, V, C, H, W), raw_gaussians.shape

    if "nc" not in _CACHE:
        _CACHE["nc"] = _build_nc()
    nc = _CACHE["nc"]

    in_maps = [
        _per_view_inputs(raw_gaussians[0, vi], extrinsics[0, vi],
                         intrinsics[0, vi])
        for vi in range(V)
    ]

    from concourse.bass_utils import run_bass_kernel_spmd

    kwargs = {}
    if _trace:
        kwargs.update(trace=True, **(_trace_kwargs or {}))
    res = run_bass_kernel_spmd(nc, in_maps, core_ids=list(range(V)), **kwargs)

    out = np.empty((V, H, W, 15), np.float32)
    for vi in range(V):
        ro = res.results[vi]["rout"].astype(np.float32)
        ro = ro.reshape(NCHUNK, P, 15, W)
        ov = out[vi]
        ov[..., SLOT_TO_REF] = ro.transpose(0, 1, 3, 2).reshape(H, W, 15)
    if _trace:
        _CACHE["last_results"] = res
    return out


# revision 8
# speedup vs baseline: 1.0679x; 1.0679x over previous
"""Trainium2 Bass kernel for nn_DecoderSplatting (v3).

Per-pixel gaussian-splat decoding over (8 views, 480x640), one view per
NeuronCore (8 cores, SPMD).  Key design (vs the v1 baseline at 460us
NTFF / ~7.5k DMA descriptors):

- HOST-side reformat: input per view -> rin[4, 120, 14*640] (chunk-major,
  partition p = h-row 120k+p, channels planar in the free dim) so each
  per-chunk DMA is one 35840-byte contiguous run per partition = 120
  descriptors.  Output -> rout[4, 120, 15*640] fp16, planar channel
  layout (contiguous stores, f32->f16 cast in the engine write path,
  host converts/permutes back).  gx/gy/consts merged into one [120, 676]
  tensor.  Total ~1.1k descriptors and ~26.6MB of HBM traffic.
- Engine balance (HW-measured costs): ACT does transcendentals + the
  per-partition-scalar Copy ops (scale/bias APs); DVE does the
  scalar-pointer mult-add chains and min/max/is_equal (Pool rejects
  those ALU ops); GpSimd does plain mult/add/sub tensor-tensor work.
- Stride-0 broadcast APs fuse the quat output scaling into one 4W-wide
  op and the means*sfac into one 3W-wide op.

Math (validated against the jax reference, rel ~7e-3 dominated by means
cancellation, same as baseline):
- e = [sig(ox)+gx, r11*sig(oy)+gy, c2]; depth = 1/(K1*sig(disp)+K2)
  means = t + (R@e) * exp(-0.5*(2*(ln(K2*E+K1+K2)-ln(1+E)) + ln|e|^2)),
  E = exp(-disp)  (the 1/A00 scale of inv(K) cancels in e/|e|)
- world quat wq = M_E @ q_cam (4x4 const per view), normalized, sign
  flipped iff min(wq)^2 == max(wq_i^2) (scipy/Shepperd pivot sign).
- All ACT transcendentals use only {Exp, Ln} => a single activation
  table set (natural_log_exp_and_others), no table thrash.
"""

import sys

import numpy as np

try:
    import concourse.bass as bass
except ImportError:  # pragma: no cover
    sys.path.insert(0, "/opt/trn_rl_repo")
    import concourse.bass as bass

import concourse.bacc as bacc

import concourse.mybir as mybir
from concourse.tile import TileContext

F32 = mybir.dt.float32
F16 = mybir.dt.float16
Alu = mybir.AluOpType
Act = mybir.ActivationFunctionType

NEAR, FAR = 0.05, 20.0
K1 = float(1.0 / NEAR - 1.0 / FAR)
K2 = float(1.0 / FAR)

V = 8
C = 14
H = 480
W = 640
P = 120          # partitions per chunk
NCHUNK = H // P  # 4
NCST = 32
CFW = W + NCHUNK + NCST  # gx | gy columns | scalar consts

# input channel permutation (raw -> kernel order)
#   raw:  [r,g,b, disp, opac, s0,s1,s2, qx,qy,qz,qw, ox,oy]
#   kern: [r,g,b, s0,s1,s2, disp, opac, qx,qy,qz,qw, ox,oy]
IN_PERM = [0, 1, 2, 5, 6, 7, 3, 4, 8, 9, 10, 11, 12, 13]
# output slot s in the kernel -> channel in the reference layout
SLOT_TO_REF = [4, 5, 6, 8, 9, 10, 7, 3, 0, 1, 2, 11, 12, 13, 14]

_CACHE = {}


class _CoveringSetBacc(bacc.Bacc):
    """Bacc whose act-table-load pass collapses to one covering table set.

    The stock pass assigns each activation the *first* table set containing
    its function (Exp -> exp_and_others, Ln -> natural_log), which ping-pongs
    a ~2.7us table load before nearly every activation.  All functions used
    here live in natural_log_exp_and_others, so rewrite every load to that
    covering set and drop the duplicates (the loads carry no sync info).
    """

    def insert_act_table_loads(self):
        super().insert_act_table_loads()
        from concourse.hw_specs import get_activation_tables

        tables = list(get_activation_tables(self.m.arch).items())
        used = set()
        for b in self.main_func.blocks:
            for i in b.instructions:
                if isinstance(i, mybir.InstActivation):
                    used.add(i.func)
        cover = None
        for idx, (_, funcs) in enumerate(tables):
            if used <= funcs:
                cover = idx
                break
        if cover is None:
            return
        for b in self.main_func.blocks:
            seen = False
            keep = []
            for i in b.instructions:
                if isinstance(i, mybir.InstLoadActFuncSet):
                    if seen:
                        continue
                    i.act_func_set_id = cover
                    seen = True
                keep.append(i)
            b.instructions[:] = keep


def _build_nc():
    nc = _CoveringSetBacc()
    rin = nc.dram_tensor("rin", [NCHUNK, P, C * W], F32, kind="ExternalInput")
    cfu = nc.dram_tensor("cfu", [P, CFW], F32, kind="ExternalInput")
    rout = nc.dram_tensor("rout", [NCHUNK, P, 15 * W], F16,
                          kind="ExternalOutput")

    va = nc.vector
    ae = nc.scalar
    ge = nc.gpsimd

    with TileContext(nc) as tc:
        with (
            tc.tile_pool(name="inp", bufs=2) as in_pool,
            tc.tile_pool(name="outp", bufs=2) as out_pool,
            tc.tile_pool(name="scr", bufs=2) as scr_pool,
            tc.tile_pool(name="consts", bufs=1) as cst_pool,
        ):
            cf = cst_pool.tile([P, CFW], F32, tag="cf", name="cf")
            nc.sync.dma_start(out=cf[:], in_=cfu[:])
            gx = cf[:, 0:W]

            def GY(k):
                return cf[:, W + k:W + k + 1]

            def CST(i):
                return cf[:, W + NCHUNK + i:W + NCHUNK + i + 1]

            def sl(t, a, b):
                return t[:, a * W:b * W]

            for k in range(NCHUNK):
                ein = nc.sync if k % 2 == 0 else nc.scalar
                eout = nc.scalar if k % 2 == 0 else nc.sync
                IT = in_pool.tile([P, C * W], F32, tag="IT", name="IT")
                ein.dma_start(out=IT[:], in_=rin[k])
                OT = out_pool.tile([P, 15 * W], F16, tag="OT", name="OT")
                # double-buffered scratch so consecutive chunks pipeline
                W4 = scr_pool.tile([P, 4 * W], F32, tag="W4", name="W4")
                W3 = scr_pool.tile([P, 3 * W], F32, tag="W3", name="W3")
                E01 = scr_pool.tile([P, 2 * W], F32, tag="E01", name="E01")
                S5 = scr_pool.tile([P, 5 * W], F32, tag="S5", name="S5")
                la, sfac, smn, isv, wb = (sl(S5, i, i + 1) for i in range(5))
                sfacb = sfac.rearrange("p (o w) -> p o w", o=1) \
                    .broadcast_to((P, 3, W))
                isvb = isv.rearrange("p (o w) -> p o w", o=1) \
                    .broadcast_to((P, 4, W))

                def it(a, b):
                    return IT[:, a * W:b * W]

                def ot(a, b):
                    return OT[:, a * W:b * W]

                # --- quat matvec first (longest chain): wq_i = M[i] @ q
                #     (start on ACT via Copy-with-scale, accumulate on DVE)
                for i in range(4):
                    wqi = sl(W4, i, i + 1)
                    ae.activation(wqi, it(8, 9), Act.Copy,
                                  scale=CST(15 + 4 * i))
                    for j in range(1, 4):
                        va.scalar_tensor_tensor(wqi, it(8 + j, 9 + j),
                                                CST(15 + 4 * i + j), wqi,
                                                Alu.mult, Alu.add)

                # --- softplus(rgb+scales) in place, 6 channels wide
                ae.activation(it(0, 6), it(0, 6), Act.Exp)
                ae.activation(ot(0, 6), it(0, 6), Act.Ln, bias=1.0)
                ae.activation(ot(3, 6), ot(3, 6), Act.Identity, scale=CST(14))

                # --- disp/opacity (in place in IT[6:8]):
                #     E = exp(-x); la = ln(K2*E+K1+K2); L = ln(1+E);
                #     opac = exp(-L_op)
                ae.activation(it(6, 8), it(6, 8), Act.Exp, scale=-1.0)
                ae.activation(la, it(6, 7), Act.Ln, scale=K2, bias=CST(31))
                ae.activation(it(6, 8), it(6, 8), Act.Ln, bias=1.0)
                ae.activation(ot(6, 7), it(7, 8), Act.Exp, scale=-1.0)

                # --- xy sigmoid chain (in place in IT[12:14]), ray e0/e1
                ae.activation(it(12, 14), it(12, 14), Act.Exp, scale=-1.0)
                ae.activation(it(12, 14), it(12, 14), Act.Ln, bias=1.0)
                ae.activation(it(12, 14), it(12, 14), Act.Exp, scale=-1.0)
                ge.tensor_tensor(sl(E01, 0, 1), it(12, 13), gx, Alu.add)
                ae.activation(sl(E01, 1, 2), it(13, 14), Act.Identity,
                              scale=CST(1), bias=GY(k))

                # --- |e|^2 and ln of it (into dead IT[12:14])
                ge.tensor_tensor(it(12, 14), E01[:], E01[:], Alu.mult)
                ge.tensor_tensor(it(12, 13), it(12, 13), it(13, 14), Alu.add)
                ae.activation(it(13, 14), it(12, 13), Act.Ln, bias=CST(0))

                # --- ld = la - ln(1+E); arg = 2*ld + ln|e|^2; sfac
                ge.tensor_tensor(la, la, it(6, 7), Alu.subtract)
                va.scalar_tensor_tensor(la, la, 2.0, it(13, 14),
                                        Alu.mult, Alu.add)
                ae.activation(sfac, la, Act.Exp, scale=-0.5)

                # --- means: m_i = (e0*Ri0 + (e1*Ri1 + Ri2c2))*sfac + t_i
                for i in range(3):
                    va.tensor_scalar(sl(W3, i, i + 1), sl(E01, 1, 2),
                                     CST(5 + i), CST(8 + i),
                                     Alu.mult, Alu.add)
                    va.scalar_tensor_tensor(sl(W3, i, i + 1), sl(E01, 0, 1),
                                            CST(2 + i), sl(W3, i, i + 1),
                                            Alu.mult, Alu.add)
                ge.tensor_tensor(
                    W3[:].rearrange("p (i w) -> p i w", i=3),
                    W3[:].rearrange("p (i w) -> p i w", i=3),
                    sfacb, Alu.mult)
                for i in range(3):
                    ae.activation(ot(8 + i, 9 + i), sl(W3, i, i + 1),
                                  Act.Identity, bias=CST(11 + i))

                # --- quat norm + sign fix
                # squares into dead IT[8:12]; m12 into dead IT[12:14]
                # (Pool TT supports only add/sub/mult -> min/max/is_equal
                #  run on DVE)
                ge.tensor_tensor(it(8, 12), W4[:], W4[:], Alu.mult)
                va.tensor_tensor(it(12, 14), it(8, 10), it(10, 12), Alu.max)
                ge.tensor_tensor(it(8, 10), it(8, 10), it(10, 12), Alu.add)
                ge.tensor_tensor(it(8, 9), it(8, 9), it(9, 10), Alu.add)
                va.tensor_tensor(it(12, 13), it(12, 13), it(13, 14), Alu.max)
                va.tensor_tensor(it(10, 12), sl(W4, 0, 2), sl(W4, 2, 4),
                                 Alu.min)
                va.tensor_tensor(it(10, 11), it(10, 11), it(11, 12), Alu.min)
                ge.tensor_tensor(smn, it(10, 11), it(10, 11), Alu.mult)
                ae.activation(it(9, 10), it(8, 9), Act.Ln)
                ae.activation(isv, it(9, 10), Act.Exp, scale=-0.5)
                va.tensor_tensor(smn, smn, it(12, 13), Alu.is_equal)
                # isv *= sign: isv - 2*isv*iseq
                ge.tensor_tensor(wb, isv, smn, Alu.mult)
                va.scalar_tensor_tensor(isv, wb, -2.0, isv,
                                        Alu.mult, Alu.add)
                va.tensor_tensor(
                    OT[:, 11 * W:15 * W].rearrange("p (i w) -> p i w", i=4),
                    W4[:].rearrange("p (i w) -> p i w", i=4),
                    isvb, Alu.mult)

                ge.memset(ot(7, 8), 1.0)

                eout.dma_start(out=rout[k], in_=OT[:])
    nc.finalize()
    return nc


def _mat_to_quat_wxyz(m):
    m = np.asarray(m, np.float64)
    m00, m01, m02 = m[0, 0], m[0, 1], m[0, 2]
    m10, m11, m12 = m[1, 0], m[1, 1], m[1, 2]
    m20, m21, m22 = m[2, 0], m[2, 1], m[2, 2]
    tr = m00 + m11 + m22
    qs = [
        np.array([m21 - m12, 1 + m00 - m11 - m22, m01 + m10, m02 + m20]),
        np.array([m02 - m20, m01 + m10, 1 + m11 - m00 - m22, m12 + m21]),
        np.array([m10 - m01, m02 + m20, m12 + m21, 1 + m22 - m00 - m11]),
        np.array([1 + tr, m21 - m12, m02 - m20, m10 - m01]),
    ]
    q = qs[int(np.argmax([m00, m11, m22, tr]))]
    return q / np.linalg.norm(q)


def _per_view_inputs(raw_v, E, K):
    """Host-side reformat + per-view constants -> the in_map for one core."""
    A = np.linalg.inv(K.astype(np.float32))
    a00 = float(A[0, 0])
    assert a00 > 0
    assert abs(A[0, 1]) < 1e-6 * a00 and abs(A[1, 0]) < 1e-6 * a00
    assert abs(A[2, 0]) < 1e-9 and abs(A[2, 1]) < 1e-9
    assert np.allclose(E[3], [0, 0, 0, 1], atol=1e-6)
    R = E[:3, :3].astype(np.float64)
    t = E[:3, 3].astype(np.float64)
    c2 = float(A[2, 2]) / a00
    r11 = float(A[1, 1]) / a00
    mult = float(np.linalg.inv(K[:2, :2].astype(np.float32)).sum())

    ew, ex, ey, ez = _mat_to_quat_wxyz(R)
    M = np.array([
        [-ex, -ey, -ez, ew],
        [ew, -ez, ey, ex],
        [ez, ew, -ex, ey],
        [-ey, ex, ew, ez],
    ], np.float64)

    cstv = np.zeros(NCST, np.float64)
    cstv[0] = c2 * c2
    cstv[1] = r11
    for i in range(3):
        cstv[2 + i] = R[i, 0]
        cstv[5 + i] = R[i, 1]
        cstv[8 + i] = R[i, 2] * c2
        cstv[11 + i] = t[i]
    cstv[14] = mult
    cstv[15:31] = M.reshape(-1)
    cstv[31] = K1 + K2   # bias for ln(k2*E + (k1+k2))

    xs = np.arange(W, dtype=np.float32)
    gxrow = (xs - np.float32(0.5)) + np.float32(float(A[0, 2]) / a00)
    ys = np.arange(H, dtype=np.float32)
    gycol = np.float32(r11) * (ys - np.float32(0.5)) + \
        np.float32(float(A[1, 2]) / a00)
    gyt = gycol.reshape(NCHUNK, P).T  # [P, NCHUNK]

    cfu = np.empty((P, CFW), np.float32)
    cfu[:, 0:W] = gxrow[None, :]
    cfu[:, W:W + NCHUNK] = gyt
    cfu[:, W + NCHUNK:] = cstv.astype(np.float32)[None, :]

    # [14, 480, 640] -> perm channels -> [4, 120, 14, 640] -> flat free dim
    rin = np.ascontiguousarray(
        raw_v[IN_PERM].reshape(C, NCHUNK, P, W).transpose(1, 2, 0, 3)
    ).reshape(NCHUNK, P, C * W)

    return {"rin": rin, "cfu": cfu}


def kernel(raw_gaussians, extrinsics, intrinsics, _trace=False,
           _trace_kwargs=None):
    raw_gaussians = np.asarray(raw_gaussians, np.float32)
    extrinsics = np.asarray(extrinsics, np.float32)
    intrinsics = np.asarray(intrinsics, np.float32)
    b, v, c, h, w = raw_gaussians.shape
    assert (b, v, c, h, w) == (1, V, C, H, W), raw_gaussians.shape

    if "nc" not in _CACHE:
        _CACHE["nc"] = _build_nc()
    nc = _CACHE["nc"]

    in_maps = [
        _per_view_inputs(raw_gaussians[0, vi], extrinsics[0, vi],
                         intrinsics[0, vi])
        for vi in range(V)
    ]

    from concourse.bass_utils import run_bass_kernel_spmd

    kwargs = {}
    if _trace:
        kwargs.update(trace=True, **(_trace_kwargs or {}))
    res = run_bass_kernel_spmd(nc, in_maps, core_ids=list(range(V)), **kwargs)

    out = np.empty((V, H, W, 15), np.float32)
    for vi in range(V):
        ro = res.results[vi]["rout"].astype(np.float32)
        ro = ro.reshape(NCHUNK, P, 15, W)
        ov = out[vi]
        ov[..., SLOT_TO_REF] = ro.transpose(0, 1, 3, 2).reshape(H, W, 15)
    if _trace:
        _CACHE["last_results"] = res
    return out


# revision 9
# speedup vs baseline: 1.1106x; 1.0400x over previous
"""Trainium2 Bass kernel for nn_DecoderSplatting (v3).

Per-pixel gaussian-splat decoding over (8 views, 480x640), one view per
NeuronCore (8 cores, SPMD).  Key design (vs the v1 baseline at 460us
NTFF / ~7.5k DMA descriptors):

- HOST-side reformat: input per view -> rin[4, 120, 14*640] (chunk-major,
  partition p = h-row 120k+p, channels planar in the free dim) so each
  per-chunk DMA is one 35840-byte contiguous run per partition = 120
  descriptors.  Output -> rout[4, 120, 15*640] fp16, planar channel
  layout (contiguous stores, f32->f16 cast in the engine write path,
  host converts/permutes back).  gx/gy/consts merged into one [120, 676]
  tensor.  Total ~1.1k descriptors and ~26.6MB of HBM traffic.
- Engine balance (HW-measured costs): ACT does transcendentals + the
  per-partition-scalar Copy ops (scale/bias APs); DVE does the
  scalar-pointer mult-add chains and min/max/is_equal (Pool rejects
  those ALU ops); GpSimd does plain mult/add/sub tensor-tensor work.
- Stride-0 broadcast APs fuse the quat output scaling into one 4W-wide
  op and the means*sfac into one 3W-wide op.

Math (validated against the jax reference, rel ~7e-3 dominated by means
cancellation, same as baseline):
- e = [sig(ox)+gx, r11*sig(oy)+gy, c2]; depth = 1/(K1*sig(disp)+K2)
  means = t + (R@e) * exp(-0.5*(2*(ln(K2*E+K1+K2)-ln(1+E)) + ln|e|^2)),
  E = exp(-disp)  (the 1/A00 scale of inv(K) cancels in e/|e|)
- world quat wq = M_E @ q_cam (4x4 const per view), normalized, sign
  flipped iff min(wq)^2 == max(wq_i^2) (scipy/Shepperd pivot sign).
- All ACT transcendentals use only {Exp, Ln} => a single activation
  table set (natural_log_exp_and_others), no table thrash.
"""

import sys

import numpy as np

try:
    import concourse.bass as bass
except ImportError:  # pragma: no cover
    sys.path.insert(0, "/opt/trn_rl_repo")
    import concourse.bass as bass

import concourse.bacc as bacc

import concourse.mybir as mybir
from concourse.tile import TileContext

F32 = mybir.dt.float32
F16 = mybir.dt.float16
Alu = mybir.AluOpType
Act = mybir.ActivationFunctionType

NEAR, FAR = 0.05, 20.0
K1 = float(1.0 / NEAR - 1.0 / FAR)
K2 = float(1.0 / FAR)

V = 8
C = 14
H = 480
W = 640
P = 120          # partitions per chunk
NCHUNK = H // P  # 4
NCST = 32
CFW = W + NCHUNK + NCST  # gx | gy columns | scalar consts

# input channel permutation (raw -> kernel order)
#   raw:  [r,g,b, disp, opac, s0,s1,s2, qx,qy,qz,qw, ox,oy]
#   kern: [r,g,b, s0,s1,s2, disp, opac, qx,qy,qz,qw, ox,oy]
IN_PERM = [0, 1, 2, 5, 6, 7, 3, 4, 8, 9, 10, 11, 12, 13]
# output slot s in the kernel -> channel in the reference layout
SLOT_TO_REF = [4, 5, 6, 8, 9, 10, 7, 3, 0, 1, 2, 11, 12, 13, 14]

_CACHE = {}


class _CoveringSetBacc(bacc.Bacc):
    """Bacc whose act-table-load pass collapses to one covering table set.

    The stock pass assigns each activation the *first* table set containing
    its function (Exp -> exp_and_others, Ln -> natural_log), which ping-pongs
    a ~2.7us table load before nearly every activation.  All functions used
    here live in natural_log_exp_and_others, so rewrite every load to that
    covering set and drop the duplicates (the loads carry no sync info).
    """

    def insert_act_table_loads(self):
        super().insert_act_table_loads()
        from concourse.hw_specs import get_activation_tables

        tables = list(get_activation_tables(self.m.arch).items())
        used = set()
        for b in self.main_func.blocks:
            for i in b.instructions:
                if isinstance(i, mybir.InstActivation):
                    used.add(i.func)
        cover = None
        for idx, (_, funcs) in enumerate(tables):
            if used <= funcs:
                cover = idx
                break
        if cover is None:
            return
        for b in self.main_func.blocks:
            seen = False
            keep = []
            for i in b.instructions:
                if isinstance(i, mybir.InstLoadActFuncSet):
                    if seen:
                        continue
                    i.act_func_set_id = cover
                    seen = True
                keep.append(i)
            b.instructions[:] = keep


def _build_nc():
    nc = _CoveringSetBacc()
    rin = nc.dram_tensor("rin", [NCHUNK, P, C * W], F32, kind="ExternalInput")
    cfu = nc.dram_tensor("cfu", [P, CFW], F32, kind="ExternalInput")
    rout = nc.dram_tensor("rout", [NCHUNK, P, 15 * W], F16,
                          kind="ExternalOutput")

    va = nc.vector
    ae = nc.scalar
    ge = nc.gpsimd

    with TileContext(nc) as tc:
        with (
            tc.tile_pool(name="inp", bufs=2) as in_pool,
            tc.tile_pool(name="outp", bufs=2) as out_pool,
            tc.tile_pool(name="scr", bufs=2) as scr_pool,
            tc.tile_pool(name="consts", bufs=1) as cst_pool,
        ):
            cf = cst_pool.tile([P, CFW], F32, tag="cf", name="cf")
            nc.sync.dma_start(out=cf[:], in_=cfu[:])
            gx = cf[:, 0:W]

            def GY(k):
                return cf[:, W + k:W + k + 1]

            def CST(i):
                return cf[:, W + NCHUNK + i:W + NCHUNK + i + 1]

            def sl(t, a, b):
                return t[:, a * W:b * W]

            tiles = {}

            def stage1(k):
                """Load + matvec + transcendentals + rays (depends only on
                this chunk's input)."""
                ein = nc.sync if k % 2 == 0 else nc.scalar
                IT = in_pool.tile([P, C * W], F32, tag="IT", name="IT")
                ein.dma_start(out=IT[:], in_=rin[k])
                OT = out_pool.tile([P, 15 * W], F16, tag="OT", name="OT")
                W4 = scr_pool.tile([P, 4 * W], F32, tag="W4", name="W4")
                W3 = scr_pool.tile([P, 3 * W], F32, tag="W3", name="W3")
                E01 = scr_pool.tile([P, 2 * W], F32, tag="E01", name="E01")
                S5 = scr_pool.tile([P, 5 * W], F32, tag="S5", name="S5")
                tiles[k] = (IT, OT, W4, W3, E01, S5)
                la = sl(S5, 0, 1)

                def it(a, b):
                    return IT[:, a * W:b * W]

                def ot(a, b):
                    return OT[:, a * W:b * W]

                # quat matvec (longest chain first): wq_i = M[i] @ q
                for i in range(4):
                    wqi = sl(W4, i, i + 1)
                    ae.activation(wqi, it(8, 9), Act.Copy,
                                  scale=CST(15 + 4 * i))
                    for j in range(1, 4):
                        va.scalar_tensor_tensor(wqi, it(8 + j, 9 + j),
                                                CST(15 + 4 * i + j), wqi,
                                                Alu.mult, Alu.add)

                # softplus(rgb+scales) in place, 6 channels wide
                ae.activation(it(0, 6), it(0, 6), Act.Exp)
                ae.activation(ot(0, 6), it(0, 6), Act.Ln, bias=1.0)
                ae.activation(ot(3, 6), ot(3, 6), Act.Identity, scale=CST(14))

                # disp/opacity (in place in IT[6:8])
                ae.activation(it(6, 8), it(6, 8), Act.Exp, scale=-1.0)
                ae.activation(la, it(6, 7), Act.Ln, scale=K2, bias=CST(31))
                ae.activation(it(6, 8), it(6, 8), Act.Ln, bias=1.0)
                ae.activation(ot(6, 7), it(7, 8), Act.Exp, scale=-1.0)

                # xy sigmoid chain (in place in IT[12:14]), ray e0/e1
                ae.activation(it(12, 14), it(12, 14), Act.Exp, scale=-1.0)
                ae.activation(it(12, 14), it(12, 14), Act.Ln, bias=1.0)
                ae.activation(it(12, 14), it(12, 14), Act.Exp, scale=-1.0)
                ge.tensor_tensor(sl(E01, 0, 1), it(12, 13), gx, Alu.add)
                ae.activation(sl(E01, 1, 2), it(13, 14), Act.Identity,
                              scale=CST(1), bias=GY(k))

                # |e|^2 and ln of it (into dead IT[12:14])
                ge.tensor_tensor(it(12, 14), E01[:], E01[:], Alu.mult)
                ge.tensor_tensor(it(12, 13), it(12, 13), it(13, 14), Alu.add)
                ae.activation(it(13, 14), it(12, 13), Act.Ln, bias=CST(0))

                # ld = la - ln(1+E); arg = 2*ld + ln|e|^2; sfac
                ge.tensor_tensor(la, la, it(6, 7), Alu.subtract)
                va.scalar_tensor_tensor(la, la, 2.0, it(13, 14),
                                        Alu.mult, Alu.add)
                ae.activation(sl(S5, 1, 2), la, Act.Exp, scale=-0.5)
                ge.memset(ot(7, 8), 1.0)

            def stage2(k):
                """Means + quat normalize/sign + store (tail)."""
                eout = nc.scalar if k % 2 == 0 else nc.sync
                IT, OT, W4, W3, E01, S5 = tiles.pop(k)
                sfac, smn, isv, wb = (sl(S5, i, i + 1) for i in range(1, 5))
                sfacb = sfac.rearrange("p (o w) -> p o w", o=1) \
                    .broadcast_to((P, 3, W))
                isvb = isv.rearrange("p (o w) -> p o w", o=1) \
                    .broadcast_to((P, 4, W))

                def it(a, b):
                    return IT[:, a * W:b * W]

                def ot(a, b):
                    return OT[:, a * W:b * W]

                # means: m_i = (e0*Ri0 + (e1*Ri1 + Ri2c2))*sfac + t_i
                for i in range(3):
                    va.tensor_scalar(sl(W3, i, i + 1), sl(E01, 1, 2),
                                     CST(5 + i), CST(8 + i),
                                     Alu.mult, Alu.add)
                    va.scalar_tensor_tensor(sl(W3, i, i + 1), sl(E01, 0, 1),
                                            CST(2 + i), sl(W3, i, i + 1),
                                            Alu.mult, Alu.add)
                ge.tensor_tensor(
                    W3[:].rearrange("p (i w) -> p i w", i=3),
                    W3[:].rearrange("p (i w) -> p i w", i=3),
                    sfacb, Alu.mult)
                for i in range(3):
                    ae.activation(ot(8 + i, 9 + i), sl(W3, i, i + 1),
                                  Act.Identity, bias=CST(11 + i))

                # quat norm + sign fix; squares into dead IT[8:12],
                # m12 into dead IT[12:14].  (Pool TT supports only
                # add/sub/mult -> min/max/is_equal run on DVE)
                ge.tensor_tensor(it(8, 12), W4[:], W4[:], Alu.mult)
                va.tensor_tensor(it(12, 14), it(8, 10), it(10, 12), Alu.max)
                ge.tensor_tensor(it(8, 10), it(8, 10), it(10, 12), Alu.add)
                ge.tensor_tensor(it(8, 9), it(8, 9), it(9, 10), Alu.add)
                va.tensor_tensor(it(12, 13), it(12, 13), it(13, 14), Alu.max)
                va.tensor_tensor(it(10, 12), sl(W4, 0, 2), sl(W4, 2, 4),
                                 Alu.min)
                va.tensor_tensor(it(10, 11), it(10, 11), it(11, 12), Alu.min)
                ge.tensor_tensor(smn, it(10, 11), it(10, 11), Alu.mult)
                ae.activation(it(9, 10), it(8, 9), Act.Ln)
                ae.activation(isv, it(9, 10), Act.Exp, scale=-0.5)
                va.tensor_tensor(smn, smn, it(12, 13), Alu.is_equal)
                # isv *= sign: isv - 2*isv*iseq
                ge.tensor_tensor(wb, isv, smn, Alu.mult)
                va.scalar_tensor_tensor(isv, wb, -2.0, isv,
                                        Alu.mult, Alu.add)
                va.tensor_tensor(
                    OT[:, 11 * W:15 * W].rearrange("p (i w) -> p i w", i=4),
                    W4[:].rearrange("p (i w) -> p i w", i=4),
                    isvb, Alu.mult)

                eout.dma_start(out=rout[k], in_=OT[:])

            # software pipeline: chunk k+1's head is emitted before chunk
            # k's tail so it can fill the engine queues while the tail
            # waits on its cross-engine chain
            stage1(0)
            for k in range(1, NCHUNK):
                stage1(k)
                stage2(k - 1)
            stage2(NCHUNK - 1)
    nc.finalize()
    return nc


def _mat_to_quat_wxyz(m):
    m = np.asarray(m, np.float64)
    m00, m01, m02 = m[0, 0], m[0, 1], m[0, 2]
    m10, m11, m12 = m[1, 0], m[1, 1], m[1, 2]
    m20, m21, m22 = m[2, 0], m[2, 1], m[2, 2]
    tr = m00 + m11 + m22
    qs = [
        np.array([m21 - m12, 1 + m00 - m11 - m22, m01 + m10, m02 + m20]),
        np.array([m02 - m20, m01 + m10, 1 + m11 - m00 - m22, m12 + m21]),
        np.array([m10 - m01, m02 + m20, m12 + m21, 1 + m22 - m00 - m11]),
        np.array([1 + tr, m21 - m12, m02 - m20, m10 - m01]),
    ]
    q = qs[int(np.argmax([m00, m11, m22, tr]))]
    return q / np.linalg.norm(q)


def _per_view_inputs(raw_v, E, K):
    """Host-side reformat + per-view constants -> the in_map for one core."""
    A = np.linalg.inv(K.astype(np.float32))
    a00 = float(A[0, 0])
    assert a00 > 0
    assert abs(A[0, 1]) < 1e-6 * a00 and abs(A[1, 0]) < 1e-6 * a00
    assert abs(A[2, 0]) < 1e-9 and abs(A[2, 1]) < 1e-9
    assert np.allclose(E[3], [0, 0, 0, 1], atol=1e-6)
    R = E[:3, :3].astype(np.float64)
    t = E[:3, 3].astype(np.float64)
    c2 = float(A[2, 2]) / a00
    r11 = float(A[1, 1]) / a00
    mult = float(np.linalg.inv(K[:2, :2].astype(np.float32)).sum())

    ew, ex, ey, ez = _mat_to_quat_wxyz(R)
    M = np.array([
        [-ex, -ey, -ez, ew],
        [ew, -ez, ey, ex],
        [ez, ew, -ex, ey],
        [-ey, ex, ew, ez],
    ], np.float64)

    cstv = np.zeros(NCST, np.float64)
    cstv[0] = c2 * c2
    cstv[1] = r11
    for i in range(3):
        cstv[2 + i] = R[i, 0]
        cstv[5 + i] = R[i, 1]
        cstv[8 + i] = R[i, 2] * c2
        cstv[11 + i] = t[i]
    cstv[14] = mult
    cstv[15:31] = M.reshape(-1)
    cstv[31] = K1 + K2   # bias for ln(k2*E + (k1+k2))

    xs = np.arange(W, dtype=np.float32)
    gxrow = (xs - np.float32(0.5)) + np.float32(float(A[0, 2]) / a00)
    ys = np.arange(H, dtype=np.float32)
    gycol = np.float32(r11) * (ys - np.float32(0.5)) + \
        np.float32(float(A[1, 2]) / a00)
    gyt = gycol.reshape(NCHUNK, P).T  # [P, NCHUNK]

    cfu = np.empty((P, CFW), np.float32)
    cfu[:, 0:W] = gxrow[None, :]
    cfu[:, W:W + NCHUNK] = gyt
    cfu[:, W + NCHUNK:] = cstv.astype(np.float32)[None, :]

    # [14, 480, 640] -> perm channels -> [4, 120, 14, 640] -> flat free dim
    rin = np.ascontiguousarray(
        raw_v[IN_PERM].reshape(C, NCHUNK, P, W).transpose(1, 2, 0, 3)
    ).reshape(NCHUNK, P, C * W)

    return {"rin": rin, "cfu": cfu}


def kernel(raw_gaussians, extrinsics, intrinsics, _trace=False,
           _trace_kwargs=None):
    raw_gaussians = np.asarray(raw_gaussians, np.float32)
    extrinsics = np.asarray(extrinsics, np.float32)
    intrinsics = np.asarray(intrinsics, np.float32)
    b, v, c, h, w = raw_gaussians.shape
    assert (b, v, c, h, w) == (1, V, C, H, W), raw_gaussians.shape

    if "nc" not in _CACHE:
        _CACHE["nc"] = _build_nc()
    nc = _CACHE["nc"]

    in_maps = [
        _per_view_inputs(raw_gaussians[0, vi], extrinsics[0, vi],
                         intrinsics[0, vi])
        for vi in range(V)
    ]

    from concourse.bass_utils import run_bass_kernel_spmd

    kwargs = {}
    if _trace:
        kwargs.update(trace=True, **(_trace_kwargs or {}))
    res = run_bass_kernel_spmd(nc, in_maps, core_ids=list(range(V)), **kwargs)

    out = np.empty((V, H, W, 15), np.float32)
    for vi in range(V):
        ro = res.results[vi]["rout"].astype(np.float32)
        ro = ro.reshape(NCHUNK, P, 15, W)
        ov = out[vi]
        ov[..., SLOT_TO_REF] = ro.transpose(0, 1, 3, 2).reshape(H, W, 15)
    if _trace:
        _CACHE["last_results"] = res
    return out


# revision 11
# speedup vs baseline: 1.1771x; 1.0599x over previous
"""Trainium2 Bass kernel for nn_DecoderSplatting (v3).

Per-pixel gaussian-splat decoding over (8 views, 480x640), one view per
NeuronCore (8 cores, SPMD).  Key design (vs the v1 baseline at 460us
NTFF / ~7.5k DMA descriptors):

- HOST-side reformat: input per view -> rin[4, 120, 14*640] (chunk-major,
  partition p = h-row 120k+p, channels planar in the free dim) so each
  per-chunk DMA is one 35840-byte contiguous run per partition = 120
  descriptors.  Output -> rout[4, 120, 15*640] fp16, planar channel
  layout (contiguous stores, f32->f16 cast in the engine write path,
  host converts/permutes back).  gx/gy/consts merged into one [120, 676]
  tensor.  Total ~1.1k descriptors and ~26.6MB of HBM traffic.
- Engine balance (HW-measured costs): ACT does transcendentals + the
  per-partition-scalar Copy ops (scale/bias APs); DVE does the
  scalar-pointer mult-add chains and min/max/is_equal (Pool rejects
  those ALU ops); GpSimd does plain mult/add/sub tensor-tensor work.
- Stride-0 broadcast APs fuse the quat output scaling into one 4W-wide
  op and the means*sfac into one 3W-wide op.

Math (validated against the jax reference, rel ~7e-3 dominated by means
cancellation, same as baseline):
- e = [sig(ox)+gx, r11*sig(oy)+gy, c2]; depth = 1/(K1*sig(disp)+K2)
  means = t + (R@e) * exp(-0.5*(2*(ln(K2*E+K1+K2)-ln(1+E)) + ln|e|^2)),
  E = exp(-disp)  (the 1/A00 scale of inv(K) cancels in e/|e|)
- world quat wq = M_E @ q_cam (4x4 const per view), normalized, sign
  flipped iff min(wq)^2 == max(wq_i^2) (scipy/Shepperd pivot sign).
- All ACT transcendentals use only {Exp, Ln} => a single activation
  table set (natural_log_exp_and_others), no table thrash.
"""

import sys

import numpy as np

try:
    import concourse.bass as bass
except ImportError:  # pragma: no cover
    sys.path.insert(0, "/opt/trn_rl_repo")
    import concourse.bass as bass

import concourse.bacc as bacc

import concourse.mybir as mybir
from concourse.tile import TileContext

F32 = mybir.dt.float32
F16 = mybir.dt.float16
Alu = mybir.AluOpType
Act = mybir.ActivationFunctionType

NEAR, FAR = 0.05, 20.0
K1 = float(1.0 / NEAR - 1.0 / FAR)
K2 = float(1.0 / FAR)

V = 8
C = 14
H = 480
W = 640
P = 120          # partitions per chunk
NCHUNK = H // P  # 4
NCST = 32
CFW = W + NCHUNK + NCST  # gx | gy columns | scalar consts

# input channel permutation (raw -> kernel order)
#   raw:  [r,g,b, disp, opac, s0,s1,s2, qx,qy,qz,qw, ox,oy]
#   kern: [r,g,b, s0,s1,s2, disp, opac, qx,qy,qz,qw, ox,oy]
IN_PERM = [0, 1, 2, 5, 6, 7, 3, 4, 8, 9, 10, 11, 12, 13]
# output slot s in the kernel -> channel in the reference layout
SLOT_TO_REF = [4, 5, 6, 8, 9, 10, 7, 3, 0, 1, 2, 11, 12, 13, 14]

_CACHE = {}


class _CoveringSetBacc(bacc.Bacc):
    """Bacc whose act-table-load pass collapses to one covering table set.

    The stock pass assigns each activation the *first* table set containing
    its function (Exp -> exp_and_others, Ln -> natural_log), which ping-pongs
    a ~2.7us table load before nearly every activation.  All functions used
    here live in natural_log_exp_and_others, so rewrite every load to that
    covering set and drop the duplicates (the loads carry no sync info).
    """

    def insert_act_table_loads(self):
        super().insert_act_table_loads()
        from concourse.hw_specs import get_activation_tables

        tables = list(get_activation_tables(self.m.arch).items())
        used = set()
        for b in self.main_func.blocks:
            for i in b.instructions:
                if isinstance(i, mybir.InstActivation):
                    used.add(i.func)
        cover = None
        for idx, (_, funcs) in enumerate(tables):
            if used <= funcs:
                cover = idx
                break
        if cover is None:
            return
        for b in self.main_func.blocks:
            seen = False
            keep = []
            for i in b.instructions:
                if isinstance(i, mybir.InstLoadActFuncSet):
                    if seen:
                        continue
                    i.act_func_set_id = cover
                    seen = True
                keep.append(i)
            b.instructions[:] = keep


def _build_nc():
    nc = _CoveringSetBacc()
    rin = nc.dram_tensor("rin", [NCHUNK, P, C * W], F32, kind="ExternalInput")
    cfu = nc.dram_tensor("cfu", [P, CFW], F32, kind="ExternalInput")
    rout = nc.dram_tensor("rout", [NCHUNK, P, 15 * W], F16,
                          kind="ExternalOutput")

    va = nc.vector
    ae = nc.scalar
    ge = nc.gpsimd

    with TileContext(nc) as tc:
        with (
            tc.tile_pool(name="inp", bufs=2) as in_pool,
            tc.tile_pool(name="outp", bufs=2) as out_pool,
            tc.tile_pool(name="scr", bufs=2) as scr_pool,
            tc.tile_pool(name="consts", bufs=1) as cst_pool,
        ):
            cf = cst_pool.tile([P, CFW], F32, tag="cf", name="cf")
            nc.sync.dma_start(out=cf[:], in_=cfu[:])
            gx = cf[:, 0:W]

            def GY(k):
                return cf[:, W + k:W + k + 1]

            def CST(i):
                return cf[:, W + NCHUNK + i:W + NCHUNK + i + 1]

            def sl(t, a, b):
                return t[:, a * W:b * W]

            tiles = {}

            def stage1(k):
                """Load + matvec + transcendentals + rays (depends only on
                this chunk's input)."""
                # all loads FIFO on the sync ring: chunk k's load completes
                # as early as possible instead of fair-sharing with k+1's
                ein = nc.sync
                IT = in_pool.tile([P, C * W], F32, tag="IT", name="IT")
                ein.dma_start(out=IT[:], in_=rin[k])
                OT = out_pool.tile([P, 15 * W], F16, tag="OT", name="OT")
                W4 = scr_pool.tile([P, 4 * W], F32, tag="W4", name="W4")
                W3 = scr_pool.tile([P, 3 * W], F32, tag="W3", name="W3")
                E01 = scr_pool.tile([P, 2 * W], F32, tag="E01", name="E01")
                S5 = scr_pool.tile([P, 5 * W], F32, tag="S5", name="S5")
                tiles[k] = (IT, OT, W4, W3, E01, S5)
                la = sl(S5, 0, 1)

                def it(a, b):
                    return IT[:, a * W:b * W]

                def ot(a, b):
                    return OT[:, a * W:b * W]

                # quat matvec (longest chain first): wq_i = M[i] @ q
                for i in range(4):
                    wqi = sl(W4, i, i + 1)
                    ae.activation(wqi, it(8, 9), Act.Copy,
                                  scale=CST(15 + 4 * i))
                    for j in range(1, 4):
                        va.scalar_tensor_tensor(wqi, it(8 + j, 9 + j),
                                                CST(15 + 4 * i + j), wqi,
                                                Alu.mult, Alu.add)

                # softplus(rgb+scales) in place, 6 channels wide
                ae.activation(it(0, 6), it(0, 6), Act.Exp)
                ae.activation(ot(0, 6), it(0, 6), Act.Ln, bias=1.0)
                ae.activation(ot(3, 6), ot(3, 6), Act.Identity, scale=CST(14))

                # disp/opacity (in place in IT[6:8])
                ae.activation(it(6, 8), it(6, 8), Act.Exp, scale=-1.0)
                ae.activation(la, it(6, 7), Act.Ln, scale=K2, bias=CST(31))
                ae.activation(it(6, 8), it(6, 8), Act.Ln, bias=1.0)
                ae.activation(ot(6, 7), it(7, 8), Act.Exp, scale=-1.0)

                # xy sigmoid chain (in place in IT[12:14]), ray e0/e1
                ae.activation(it(12, 14), it(12, 14), Act.Exp, scale=-1.0)
                ae.activation(it(12, 14), it(12, 14), Act.Ln, bias=1.0)
                ae.activation(it(12, 14), it(12, 14), Act.Exp, scale=-1.0)
                ge.tensor_tensor(sl(E01, 0, 1), it(12, 13), gx, Alu.add)
                ae.activation(sl(E01, 1, 2), it(13, 14), Act.Identity,
                              scale=CST(1), bias=GY(k))

                # |e|^2 and ln of it (into dead IT[12:14])
                ge.tensor_tensor(it(12, 14), E01[:], E01[:], Alu.mult)
                ge.tensor_tensor(it(12, 13), it(12, 13), it(13, 14), Alu.add)
                ae.activation(it(13, 14), it(12, 13), Act.Ln, bias=CST(0))

                # ld = la - ln(1+E); arg = 2*ld + ln|e|^2; sfac
                ge.tensor_tensor(la, la, it(6, 7), Alu.subtract)
                va.scalar_tensor_tensor(la, la, 2.0, it(13, 14),
                                        Alu.mult, Alu.add)
                ae.activation(sl(S5, 1, 2), la, Act.Exp, scale=-0.5)
                ge.memset(ot(7, 8), 1.0)

            def stage2(k):
                """Means + quat normalize/sign + store (tail)."""
                eout = nc.scalar
                IT, OT, W4, W3, E01, S5 = tiles.pop(k)
                sfac, smn, isv, wb = (sl(S5, i, i + 1) for i in range(1, 5))
                sfacb = sfac.rearrange("p (o w) -> p o w", o=1) \
                    .broadcast_to((P, 3, W))
                isvb = isv.rearrange("p (o w) -> p o w", o=1) \
                    .broadcast_to((P, 4, W))

                def it(a, b):
                    return IT[:, a * W:b * W]

                def ot(a, b):
                    return OT[:, a * W:b * W]

                # means: m_i = (e0*Ri0 + (e1*Ri1 + Ri2c2))*sfac + t_i
                for i in range(3):
                    va.tensor_scalar(sl(W3, i, i + 1), sl(E01, 1, 2),
                                     CST(5 + i), CST(8 + i),
                                     Alu.mult, Alu.add)
                    va.scalar_tensor_tensor(sl(W3, i, i + 1), sl(E01, 0, 1),
                                            CST(2 + i), sl(W3, i, i + 1),
                                            Alu.mult, Alu.add)
                ge.tensor_tensor(
                    W3[:].rearrange("p (i w) -> p i w", i=3),
                    W3[:].rearrange("p (i w) -> p i w", i=3),
                    sfacb, Alu.mult)
                for i in range(3):
                    ae.activation(ot(8 + i, 9 + i), sl(W3, i, i + 1),
                                  Act.Identity, bias=CST(11 + i))

                # quat norm + sign fix; squares into dead IT[8:12],
                # m12 into dead IT[12:14].  (Pool TT supports only
                # add/sub/mult -> min/max/is_equal run on DVE)
                ge.tensor_tensor(it(8, 12), W4[:], W4[:], Alu.mult)
                va.tensor_tensor(it(12, 14), it(8, 10), it(10, 12), Alu.max)
                ge.tensor_tensor(it(8, 10), it(8, 10), it(10, 12), Alu.add)
                ge.tensor_tensor(it(8, 9), it(8, 9), it(9, 10), Alu.add)
                va.tensor_tensor(it(12, 13), it(12, 13), it(13, 14), Alu.max)
                va.tensor_tensor(it(10, 12), sl(W4, 0, 2), sl(W4, 2, 4),
                                 Alu.min)
                va.tensor_tensor(it(10, 11), it(10, 11), it(11, 12), Alu.min)
                ge.tensor_tensor(smn, it(10, 11), it(10, 11), Alu.mult)
                ae.activation(it(9, 10), it(8, 9), Act.Ln)
                ae.activation(isv, it(9, 10), Act.Exp, scale=-0.5)
                va.tensor_tensor(smn, smn, it(12, 13), Alu.is_equal)
                # isv *= sign: isv - 2*isv*iseq
                ge.tensor_tensor(wb, isv, smn, Alu.mult)
                va.scalar_tensor_tensor(isv, wb, -2.0, isv,
                                        Alu.mult, Alu.add)
                va.tensor_tensor(
                    OT[:, 11 * W:15 * W].rearrange("p (i w) -> p i w", i=4),
                    W4[:].rearrange("p (i w) -> p i w", i=4),
                    isvb, Alu.mult)

                eout.dma_start(out=rout[k], in_=OT[:])

            # software pipeline: chunk k+1's head is emitted before chunk
            # k's tail so it can fill the engine queues while the tail
            # waits on its cross-engine chain
            stage1(0)
            for k in range(1, NCHUNK):
                stage1(k)
                stage2(k - 1)
            stage2(NCHUNK - 1)
    nc.finalize()
    return nc


def _mat_to_quat_wxyz(m):
    m = np.asarray(m, np.float64)
    m00, m01, m02 = m[0, 0], m[0, 1], m[0, 2]
    m10, m11, m12 = m[1, 0], m[1, 1], m[1, 2]
    m20, m21, m22 = m[2, 0], m[2, 1], m[2, 2]
    tr = m00 + m11 + m22
    qs = [
        np.array([m21 - m12, 1 + m00 - m11 - m22, m01 + m10, m02 + m20]),
        np.array([m02 - m20, m01 + m10, 1 + m11 - m00 - m22, m12 + m21]),
        np.array([m10 - m01, m02 + m20, m12 + m21, 1 + m22 - m00 - m11]),
        np.array([1 + tr, m21 - m12, m02 - m20, m10 - m01]),
    ]
    q = qs[int(np.argmax([m00, m11, m22, tr]))]
    return q / np.linalg.norm(q)


def _per_view_inputs(raw_v, E, K):
    """Host-side reformat + per-view constants -> the in_map for one core."""
    A = np.linalg.inv(K.astype(np.float32))
    a00 = float(A[0, 0])
    assert a00 > 0
    assert abs(A[0, 1]) < 1e-6 * a00 and abs(A[1, 0]) < 1e-6 * a00
    assert abs(A[2, 0]) < 1e-9 and abs(A[2, 1]) < 1e-9
    assert np.allclose(E[3], [0, 0, 0, 1], atol=1e-6)
    R = E[:3, :3].astype(np.float64)
    t = E[:3, 3].astype(np.float64)
    c2 = float(A[2, 2]) / a00
    r11 = float(A[1, 1]) / a00
    mult = float(np.linalg.inv(K[:2, :2].astype(np.float32)).sum())

    ew, ex, ey, ez = _mat_to_quat_wxyz(R)
    M = np.array([
        [-ex, -ey, -ez, ew],
        [ew, -ez, ey, ex],
        [ez, ew, -ex, ey],
        [-ey, ex, ew, ez],
    ], np.float64)

    cstv = np.zeros(NCST, np.float64)
    cstv[0] = c2 * c2
    cstv[1] = r11
    for i in range(3):
        cstv[2 + i] = R[i, 0]
        cstv[5 + i] = R[i, 1]
        cstv[8 + i] = R[i, 2] * c2
        cstv[11 + i] = t[i]
    cstv[14] = mult
    cstv[15:31] = M.reshape(-1)
    cstv[31] = K1 + K2   # bias for ln(k2*E + (k1+k2))

    xs = np.arange(W, dtype=np.float32)
    gxrow = (xs - np.float32(0.5)) + np.float32(float(A[0, 2]) / a00)
    ys = np.arange(H, dtype=np.float32)
    gycol = np.float32(r11) * (ys - np.float32(0.5)) + \
        np.float32(float(A[1, 2]) / a00)
    gyt = gycol.reshape(NCHUNK, P).T  # [P, NCHUNK]

    cfu = np.empty((P, CFW), np.float32)
    cfu[:, 0:W] = gxrow[None, :]
    cfu[:, W:W + NCHUNK] = gyt
    cfu[:, W + NCHUNK:] = cstv.astype(np.float32)[None, :]

    # [14, 480, 640] -> perm channels -> [4, 120, 14, 640] -> flat free dim
    rin = np.ascontiguousarray(
        raw_v[IN_PERM].reshape(C, NCHUNK, P, W).transpose(1, 2, 0, 3)
    ).reshape(NCHUNK, P, C * W)

    return {"rin": rin, "cfu": cfu}


def kernel(raw_gaussians, extrinsics, intrinsics, _trace=False,
           _trace_kwargs=None):
    raw_gaussians = np.asarray(raw_gaussians, np.float32)
    extrinsics = np.asarray(extrinsics, np.float32)
    intrinsics = np.asarray(intrinsics, np.float32)
    b, v, c, h, w = raw_gaussians.shape
    assert (b, v, c, h, w) == (1, V, C, H, W), raw_gaussians.shape

    if "nc" not in _CACHE:
        _CACHE["nc"] = _build_nc()
    nc = _CACHE["nc"]

    in_maps = [
        _per_view_inputs(raw_gaussians[0, vi], extrinsics[0, vi],
                         intrinsics[0, vi])
        for vi in range(V)
    ]

    from concourse.bass_utils import run_bass_kernel_spmd

    kwargs = {}
    if _trace:
        kwargs.update(trace=True, **(_trace_kwargs or {}))
    res = run_bass_kernel_spmd(nc, in_maps, core_ids=list(range(V)), **kwargs)

    out = np.empty((V, H, W, 15), np.float32)
    for vi in range(V):
        ro = res.results[vi]["rout"].astype(np.float32)
        ro = ro.reshape(NCHUNK, P, 15, W)
        ov = out[vi]
        ov[..., SLOT_TO_REF] = ro.transpose(0, 1, 3, 2).reshape(H, W, 15)
    if _trace:
        _CACHE["last_results"] = res
    return out


# revision 12
# speedup vs baseline: 1.1862x; 1.0077x over previous
"""Trainium2 Bass kernel for nn_DecoderSplatting (v3).

Per-pixel gaussian-splat decoding over (8 views, 480x640), one view per
NeuronCore (8 cores, SPMD).  Key design (vs the v1 baseline at 460us
NTFF / ~7.5k DMA descriptors):

- HOST-side reformat: input per view -> rin[4, 120, 14*640] (chunk-major,
  partition p = h-row 120k+p, channels planar in the free dim) so each
  per-chunk DMA is one 35840-byte contiguous run per partition = 120
  descriptors.  Output -> rout[4, 120, 15*640] fp16, planar channel
  layout (contiguous stores, f32->f16 cast in the engine write path,
  host converts/permutes back).  gx/gy/consts merged into one [120, 676]
  tensor.  Total ~1.1k descriptors and ~26.6MB of HBM traffic.
- Engine balance (HW-measured costs): ACT does transcendentals + the
  per-partition-scalar Copy ops (scale/bias APs); DVE does the
  scalar-pointer mult-add chains and min/max/is_equal (Pool rejects
  those ALU ops); GpSimd does plain mult/add/sub tensor-tensor work.
- Stride-0 broadcast APs fuse the quat output scaling into one 4W-wide
  op and the means*sfac into one 3W-wide op.

Math (validated against the jax reference, rel ~7e-3 dominated by means
cancellation, same as baseline):
- e = [sig(ox)+gx, r11*sig(oy)+gy, c2]; depth = 1/(K1*sig(disp)+K2)
  means = t + (R@e) * exp(-0.5*(2*(ln(K2*E+K1+K2)-ln(1+E)) + ln|e|^2)),
  E = exp(-disp)  (the 1/A00 scale of inv(K) cancels in e/|e|)
- world quat wq = M_E @ q_cam (4x4 const per view), normalized, sign
  flipped iff min(wq)^2 == max(wq_i^2) (scipy/Shepperd pivot sign).
- All ACT transcendentals use only {Exp, Ln} => a single activation
  table set (natural_log_exp_and_others), no table thrash.
"""

import sys

import numpy as np

try:
    import concourse.bass as bass
except ImportError:  # pragma: no cover
    sys.path.insert(0, "/opt/trn_rl_repo")
    import concourse.bass as bass

import concourse.bacc as bacc

import concourse.mybir as mybir
from concourse.tile import TileContext

F32 = mybir.dt.float32
F16 = mybir.dt.float16
Alu = mybir.AluOpType
Act = mybir.ActivationFunctionType

NEAR, FAR = 0.05, 20.0
K1 = float(1.0 / NEAR - 1.0 / FAR)
K2 = float(1.0 / FAR)

V = 8
C = 14
H = 480
W = 640
P = 120          # partitions per chunk
NCHUNK = H // P  # 4
NCST = 32
CFW = W + NCHUNK + NCST  # gx | gy columns | scalar consts

# input channel permutation (raw -> kernel order)
#   raw:  [r,g,b, disp, opac, s0,s1,s2, qx,qy,qz,qw, ox,oy]
#   kern: [r,g,b, s0,s1,s2, disp, opac, qx,qy,qz,qw, ox,oy]
IN_PERM = [0, 1, 2, 5, 6, 7, 3, 4, 8, 9, 10, 11, 12, 13]
# output slot s in the kernel -> channel in the reference layout
SLOT_TO_REF = [4, 5, 6, 8, 9, 10, 7, 3, 0, 1, 2, 11, 12, 13, 14]

_CACHE = {}


class _CoveringSetBacc(bacc.Bacc):
    """Bacc whose act-table-load pass collapses to one covering table set.

    The stock pass assigns each activation the *first* table set containing
    its function (Exp -> exp_and_others, Ln -> natural_log), which ping-pongs
    a ~2.7us table load before nearly every activation.  All functions used
    here live in natural_log_exp_and_others, so rewrite every load to that
    covering set and drop the duplicates (the loads carry no sync info).
    """

    def insert_act_table_loads(self):
        super().insert_act_table_loads()
        from concourse.hw_specs import get_activation_tables

        tables = list(get_activation_tables(self.m.arch).items())
        used = set()
        for b in self.main_func.blocks:
            for i in b.instructions:
                if isinstance(i, mybir.InstActivation):
                    used.add(i.func)
        cover = None
        for idx, (_, funcs) in enumerate(tables):
            if used <= funcs:
                cover = idx
                break
        if cover is None:
            return
        for b in self.main_func.blocks:
            seen = False
            keep = []
            for i in b.instructions:
                if isinstance(i, mybir.InstLoadActFuncSet):
                    if seen:
                        continue
                    i.act_func_set_id = cover
                    seen = True
                keep.append(i)
            b.instructions[:] = keep


def _build_nc():
    nc = _CoveringSetBacc()
    rin = nc.dram_tensor("rin", [NCHUNK, P, C * W], F32, kind="ExternalInput")
    cfu = nc.dram_tensor("cfu", [P, CFW], F32, kind="ExternalInput")
    rout = nc.dram_tensor("rout", [NCHUNK, P, 15 * W], F16,
                          kind="ExternalOutput")

    va = nc.vector
    ae = nc.scalar
    ge = nc.gpsimd

    with TileContext(nc) as tc:
        with (
            tc.tile_pool(name="inp", bufs=2) as in_pool,
            tc.tile_pool(name="outp", bufs=2) as out_pool,
            tc.tile_pool(name="scr", bufs=2) as scr_pool,
            tc.tile_pool(name="consts", bufs=1) as cst_pool,
        ):
            # cf rides the store ring: its 120 small descriptors would
            # otherwise FIFO-block the first input chunk on the load ring
            cf = cst_pool.tile([P, CFW], F32, tag="cf", name="cf")
            nc.scalar.dma_start(out=cf[:], in_=cfu[:])
            gx = cf[:, 0:W]

            def GY(k):
                return cf[:, W + k:W + k + 1]

            def CST(i):
                return cf[:, W + NCHUNK + i:W + NCHUNK + i + 1]

            def sl(t, a, b):
                return t[:, a * W:b * W]

            tiles = {}

            def stage1(k):
                """Load + matvec + transcendentals + rays (depends only on
                this chunk's input)."""
                # all loads FIFO on the sync ring: chunk k's load completes
                # as early as possible instead of fair-sharing with k+1's
                ein = nc.sync
                IT = in_pool.tile([P, C * W], F32, tag="IT", name="IT")
                ein.dma_start(out=IT[:], in_=rin[k])
                OT = out_pool.tile([P, 15 * W], F16, tag="OT", name="OT")
                W4 = scr_pool.tile([P, 4 * W], F32, tag="W4", name="W4")
                W3 = scr_pool.tile([P, 3 * W], F32, tag="W3", name="W3")
                E01 = scr_pool.tile([P, 2 * W], F32, tag="E01", name="E01")
                S5 = scr_pool.tile([P, 5 * W], F32, tag="S5", name="S5")
                tiles[k] = (IT, OT, W4, W3, E01, S5)
                la = sl(S5, 0, 1)

                def it(a, b):
                    return IT[:, a * W:b * W]

                def ot(a, b):
                    return OT[:, a * W:b * W]

                # quat matvec (longest chain first): wq_i = M[i] @ q
                for i in range(4):
                    wqi = sl(W4, i, i + 1)
                    ae.activation(wqi, it(8, 9), Act.Copy,
                                  scale=CST(15 + 4 * i))
                    for j in range(1, 4):
                        va.scalar_tensor_tensor(wqi, it(8 + j, 9 + j),
                                                CST(15 + 4 * i + j), wqi,
                                                Alu.mult, Alu.add)

                # softplus(rgb+scales) in place, 6 channels wide
                ae.activation(it(0, 6), it(0, 6), Act.Exp)
                ae.activation(ot(0, 6), it(0, 6), Act.Ln, bias=1.0)
                ae.activation(ot(3, 6), ot(3, 6), Act.Identity, scale=CST(14))

                # disp/opacity (in place in IT[6:8])
                ae.activation(it(6, 8), it(6, 8), Act.Exp, scale=-1.0)
                ae.activation(la, it(6, 7), Act.Ln, scale=K2, bias=CST(31))
                ae.activation(it(6, 8), it(6, 8), Act.Ln, bias=1.0)
                ae.activation(ot(6, 7), it(7, 8), Act.Exp, scale=-1.0)

                # xy sigmoid chain (in place in IT[12:14]), ray e0/e1
                ae.activation(it(12, 14), it(12, 14), Act.Exp, scale=-1.0)
                ae.activation(it(12, 14), it(12, 14), Act.Ln, bias=1.0)
                ae.activation(it(12, 14), it(12, 14), Act.Exp, scale=-1.0)
                ge.tensor_tensor(sl(E01, 0, 1), it(12, 13), gx, Alu.add)
                ae.activation(sl(E01, 1, 2), it(13, 14), Act.Identity,
                              scale=CST(1), bias=GY(k))

                # |e|^2 and ln of it (into dead IT[12:14])
                ge.tensor_tensor(it(12, 14), E01[:], E01[:], Alu.mult)
                ge.tensor_tensor(it(12, 13), it(12, 13), it(13, 14), Alu.add)
                ae.activation(it(13, 14), it(12, 13), Act.Ln, bias=CST(0))

                # ld = la - ln(1+E); arg = 2*ld + ln|e|^2; sfac
                ge.tensor_tensor(la, la, it(6, 7), Alu.subtract)
                va.scalar_tensor_tensor(la, la, 2.0, it(13, 14),
                                        Alu.mult, Alu.add)
                ae.activation(sl(S5, 1, 2), la, Act.Exp, scale=-0.5)
                ge.memset(ot(7, 8), 1.0)

            def stage2(k):
                """Means + quat normalize/sign + store (tail)."""
                eout = nc.scalar
                IT, OT, W4, W3, E01, S5 = tiles.pop(k)
                sfac, smn, isv, wb = (sl(S5, i, i + 1) for i in range(1, 5))
                sfacb = sfac.rearrange("p (o w) -> p o w", o=1) \
                    .broadcast_to((P, 3, W))
                isvb = isv.rearrange("p (o w) -> p o w", o=1) \
                    .broadcast_to((P, 4, W))

                def it(a, b):
                    return IT[:, a * W:b * W]

                def ot(a, b):
                    return OT[:, a * W:b * W]

                # means: m_i = (e0*Ri0 + (e1*Ri1 + Ri2c2))*sfac + t_i
                for i in range(3):
                    va.tensor_scalar(sl(W3, i, i + 1), sl(E01, 1, 2),
                                     CST(5 + i), CST(8 + i),
                                     Alu.mult, Alu.add)
                    va.scalar_tensor_tensor(sl(W3, i, i + 1), sl(E01, 0, 1),
                                            CST(2 + i), sl(W3, i, i + 1),
                                            Alu.mult, Alu.add)
                ge.tensor_tensor(
                    W3[:].rearrange("p (i w) -> p i w", i=3),
                    W3[:].rearrange("p (i w) -> p i w", i=3),
                    sfacb, Alu.mult)
                for i in range(3):
                    ae.activation(ot(8 + i, 9 + i), sl(W3, i, i + 1),
                                  Act.Identity, bias=CST(11 + i))

                # quat norm + sign fix; squares into dead IT[8:12],
                # m12 into dead IT[12:14].  (Pool TT supports only
                # add/sub/mult -> min/max/is_equal run on DVE)
                ge.tensor_tensor(it(8, 12), W4[:], W4[:], Alu.mult)
                va.tensor_tensor(it(12, 14), it(8, 10), it(10, 12), Alu.max)
                ge.tensor_tensor(it(8, 10), it(8, 10), it(10, 12), Alu.add)
                ge.tensor_tensor(it(8, 9), it(8, 9), it(9, 10), Alu.add)
                va.tensor_tensor(it(12, 13), it(12, 13), it(13, 14), Alu.max)
                va.tensor_tensor(it(10, 12), sl(W4, 0, 2), sl(W4, 2, 4),
                                 Alu.min)
                va.tensor_tensor(it(10, 11), it(10, 11), it(11, 12), Alu.min)
                ge.tensor_tensor(smn, it(10, 11), it(10, 11), Alu.mult)
                ae.activation(it(9, 10), it(8, 9), Act.Ln)
                ae.activation(isv, it(9, 10), Act.Exp, scale=-0.5)
                va.tensor_tensor(smn, smn, it(12, 13), Alu.is_equal)
                # isv *= sign: isv - 2*isv*iseq
                ge.tensor_tensor(wb, isv, smn, Alu.mult)
                va.scalar_tensor_tensor(isv, wb, -2.0, isv,
                                        Alu.mult, Alu.add)
                va.tensor_tensor(
                    OT[:, 11 * W:15 * W].rearrange("p (i w) -> p i w", i=4),
                    W4[:].rearrange("p (i w) -> p i w", i=4),
                    isvb, Alu.mult)

                eout.dma_start(out=rout[k], in_=OT[:])

            # software pipeline: chunk k+1's head is emitted before chunk
            # k's tail so it can fill the engine queues while the tail
            # waits on its cross-engine chain
            stage1(0)
            for k in range(1, NCHUNK):
                stage1(k)
                stage2(k - 1)
            stage2(NCHUNK - 1)
    nc.finalize()
    return nc


def _mat_to_quat_wxyz(m):
    m = np.asarray(m, np.float64)
    m00, m01, m02 = m[0, 0], m[0, 1], m[0, 2]
    m10, m11, m12 = m[1, 0], m[1, 1], m[1, 2]
    m20, m21, m22 = m[2, 0], m[2, 1], m[2, 2]
    tr = m00 + m11 + m22
    qs = [
        np.array([m21 - m12, 1 + m00 - m11 - m22, m01 + m10, m02 + m20]),
        np.array([m02 - m20, m01 + m10, 1 + m11 - m00 - m22, m12 + m21]),
        np.array([m10 - m01, m02 + m20, m12 + m21, 1 + m22 - m00 - m11]),
        np.array([1 + tr, m21 - m12, m02 - m20, m10 - m01]),
    ]
    q = qs[int(np.argmax([m00, m11, m22, tr]))]
    return q / np.linalg.norm(q)


def _per_view_inputs(raw_v, E, K):
    """Host-side reformat + per-view constants -> the in_map for one core."""
    A = np.linalg.inv(K.astype(np.float32))
    a00 = float(A[0, 0])
    assert a00 > 0
    assert abs(A[0, 1]) < 1e-6 * a00 and abs(A[1, 0]) < 1e-6 * a00
    assert abs(A[2, 0]) < 1e-9 and abs(A[2, 1]) < 1e-9
    assert np.allclose(E[3], [0, 0, 0, 1], atol=1e-6)
    R = E[:3, :3].astype(np.float64)
    t = E[:3, 3].astype(np.float64)
    c2 = float(A[2, 2]) / a00
    r11 = float(A[1, 1]) / a00
    mult = float(np.linalg.inv(K[:2, :2].astype(np.float32)).sum())

    ew, ex, ey, ez = _mat_to_quat_wxyz(R)
    M = np.array([
        [-ex, -ey, -ez, ew],
        [ew, -ez, ey, ex],
        [ez, ew, -ex, ey],
        [-ey, ex, ew, ez],
    ], np.float64)

    cstv = np.zeros(NCST, np.float64)
    cstv[0] = c2 * c2
    cstv[1] = r11
    for i in range(3):
        cstv[2 + i] = R[i, 0]
        cstv[5 + i] = R[i, 1]
        cstv[8 + i] = R[i, 2] * c2
        cstv[11 + i] = t[i]
    cstv[14] = mult
    cstv[15:31] = M.reshape(-1)
    cstv[31] = K1 + K2   # bias for ln(k2*E + (k1+k2))

    xs = np.arange(W, dtype=np.float32)
    gxrow = (xs - np.float32(0.5)) + np.float32(float(A[0, 2]) / a00)
    ys = np.arange(H, dtype=np.float32)
    gycol = np.float32(r11) * (ys - np.float32(0.5)) + \
        np.float32(float(A[1, 2]) / a00)
    gyt = gycol.reshape(NCHUNK, P).T  # [P, NCHUNK]

    cfu = np.empty((P, CFW), np.float32)
    cfu[:, 0:W] = gxrow[None, :]
    cfu[:, W:W + NCHUNK] = gyt
    cfu[:, W + NCHUNK:] = cstv.astype(np.float32)[None, :]

    # [14, 480, 640] -> perm channels -> [4, 120, 14, 640] -> flat free dim
    rin = np.ascontiguousarray(
        raw_v[IN_PERM].reshape(C, NCHUNK, P, W).transpose(1, 2, 0, 3)
    ).reshape(NCHUNK, P, C * W)

    return {"rin": rin, "cfu": cfu}


def kernel(raw_gaussians, extrinsics, intrinsics, _trace=False,
           _trace_kwargs=None):
    raw_gaussians = np.asarray(raw_gaussians, np.float32)
    extrinsics = np.asarray(extrinsics, np.float32)
    intrinsics = np.asarray(intrinsics, np.float32)
    b, v, c, h, w = raw_gaussians.shape
    assert (b, v, c, h, w) == (1, V, C, H, W), raw_gaussians.shape

    if "nc" not in _CACHE:
        _CACHE["nc"] = _build_nc()
    nc = _CACHE["nc"]

    in_maps = [
        _per_view_inputs(raw_gaussians[0, vi], extrinsics[0, vi],
                         intrinsics[0, vi])
        for vi in range(V)
    ]

    from concourse.bass_utils import run_bass_kernel_spmd

    kwargs = {}
    if _trace:
        kwargs.update(trace=True, **(_trace_kwargs or {}))
    res = run_bass_kernel_spmd(nc, in_maps, core_ids=list(range(V)), **kwargs)

    out = np.empty((V, H, W, 15), np.float32)
    for vi in range(V):
        ro = res.results[vi]["rout"].astype(np.float32)
        ro = ro.reshape(NCHUNK, P, 15, W)
        ov = out[vi]
        ov[..., SLOT_TO_REF] = ro.transpose(0, 1, 3, 2).reshape(H, W, 15)
    if _trace:
        _CACHE["last_results"] = res
    return out
